# revision 8
# baseline (speedup 1.0000x reference)
"""InfoNCE loss kernel for Trainium2, 8 NeuronCores.

loss = 0.5*( mean_i[ log(sum_j exp(s_ij)+eps) - s_ii ]
           + mean_j[ log(sum_i exp(s_ij)+eps) - s_jj ] ),  s = scale * img @ txt.T

Sharding: each core owns N/8 = 2048 image rows vs ALL 16384 text rows.
Per core, for each 128-row text block t, PE computes the transposed logits
block simT[t] = [128 (txt j), 2048 (img i)] with the txt block as the
stationary matmul operand, in fp8e4m3 DoubleRow mode (inputs pre-scaled by
32 on the host; the 1/1024 comes out in the exp scale).  ScalarE applies
exp (scale fused) and its accum_out gives the per-j partial column sums for
free; VectorE accumulates exp blocks into a [128, 2048] bf16 running
row-sum.  The diagonal <img_i, txt_i> partials are computed on VectorE in
slack cycles interleaved with the main loop.

No device-side collective: each core DMAs out its partials (column partial
sums, the bf16 row-sum accumulator, diagonal partials) and the final O(N)
reduction across shards (sum partials, ln, mean) runs on the host as part
of the unshard step.

Startup: the tensor engine's DVFS ramp (~first 9 matmuls run at ~0.6x
clock) is consumed by tiny warmup matmuls on memset data before the real
inputs land; the img operand is stored chunk-major ([P, NCH, KT, CH]) so
the first real matmul only waits on a 128KB contiguous piece, and the
startup loads are spread across four engine DMA queues.
"""

import numpy as np
import ml_dtypes

N = 16384
D = 512
NCORES = 8
S = N // NCORES          # 2048 image rows per core
P = 128                  # partitions
KT = D // P              # 4 contraction tiles
TB = N // P              # 128 text blocks
CH = 512                 # matmul moving-operand chunk (one PSUM bank)
NCH = S // CH            # 4 chunks
EPS = 1e-8
FS = 32.0                # fp8 pre-scale; logits carry FS*FS
H = S // 2


def _build(scale: float):
    import concourse.bacc as bacc
    import concourse.mybir as mybir
    import concourse.tile as tile

    dt = mybir.dt
    AF = mybir.ActivationFunctionType
    DR = mybir.MatmulPerfMode.DoubleRow

    nc = bacc.Bacc("TRN2", target_bir_lowering=False, debug=False,
                   num_devices=NCORES)

    A = nc.dram_tensor("img_a", [P, NCH, KT, CH], dt.float8e4,
                       kind="ExternalInput")
    T = nc.dram_tensor("txt_t", [P, NCH, KT, CH], dt.float8e4,
                       kind="ExternalInput")
    B = nc.dram_tensor("txt_b", [TB, P, KT, P], dt.float8e4,
                       kind="ExternalInput")
    out_col = nc.dram_tensor("colp", [P, TB], dt.float32,
                             kind="ExternalOutput")
    out_acc = nc.dram_tensor("accout", [P, S], dt.bfloat16,
                             kind="ExternalOutput")
    out_dvec = nc.dram_tensor("dvec", [P, NCH], dt.float32,
                              kind="ExternalOutput")

    with tile.TileContext(nc) as tc:
        with (
            tc.tile_pool(name="const", bufs=1) as cpool,
            tc.tile_pool(name="wts", bufs=4) as wpool,
            tc.tile_pool(name="expp", bufs=3) as epool,
            tc.tile_pool(name="accp", bufs=1) as apool,
            tc.tile_pool(name="small", bufs=1) as spool,
            tc.tile_pool(name="diagp", bufs=2) as gpool,
        ):
            # warmup data for the PE clock ramp: tiny matmuls on memset
            # bytes, issued before any input DMA completes
            wu = cpool.tile([P, 2], dt.bfloat16)
            nc.vector.memset(wu[:], 0.0)

            # startup loads spread over four engine DMA queues; the first
            # matmul only needs btile0 + the (c=0, k=0:2) img piece
            btile0 = wpool.tile([P, KT, P], dt.float8e4, tag="bt")
            nc.sync.dma_start(btile0[:], B[0])
            a_sb = cpool.tile([P, NCH, KT, CH], dt.float8e4)
            nc.sync.dma_start(a_sb[:, 0, 0:2, :], A[:, 0, 0:2, :])
            nc.sync.dma_start(a_sb[:, 1, 0:2, :], A[:, 1, 0:2, :])
            nc.gpsimd.dma_start(a_sb[:, 2, 0:2, :], A[:, 2, 0:2, :])
            nc.gpsimd.dma_start(a_sb[:, 3, 0:2, :], A[:, 3, 0:2, :])
            nc.gpsimd.dma_start(a_sb[:, 0, 2:4, :], A[:, 0, 2:4, :])
            nc.gpsimd.dma_start(a_sb[:, 1, 2:4, :], A[:, 1, 2:4, :])
            nc.scalar.dma_start(a_sb[:, 2, 2:4, :], A[:, 2, 2:4, :])
            nc.scalar.dma_start(a_sb[:, 3, 2:4, :], A[:, 3, 2:4, :])
            t_sb = cpool.tile([P, NCH, KT, CH], dt.float8e4)
            nc.scalar.dma_start(t_sb[:], T[:])

            acc = apool.tile([P, S], dt.bfloat16)
            nc.vector.memset(acc[:], 0.0)
            payload = spool.tile([P, TB], dt.float32)
            dvec4 = spool.tile([P, NCH], dt.float32)

            with tc.tile_pool(name="wup", bufs=1, space="PSUM") as wp:
                wu_ps = wp.tile([1, 2], dt.float32)
                for _ in range(10):
                    nc.tensor.matmul(wu_ps[:], lhsT=wu[:, 0:1], rhs=wu[:],
                                     start=True, stop=True)

            with tc.tile_pool(name="psmain", bufs=2, space="PSUM") as pp:
                for t in range(TB):
                    if t == 0:
                        btile = btile0
                    else:
                        btile = wpool.tile([P, KT, P], dt.float8e4, tag="bt")
                        nc.sync.dma_start(btile[:], B[t])
                    ps = pp.tile([P, S], dt.float32, tag="ps")
                    for k in range(0, KT, 2):
                        for c in range(NCH):
                            nc.tensor.matmul(
                                ps[:, c * CH:(c + 1) * CH],
                                lhsT=btile[:, k:k + 2, :],
                                rhs=a_sb[:, c, k:k + 2, :],
                                start=(k == 0),
                                stop=(k == KT - 2),
                                perf_mode=DR,
                            )
                    ex = epool.tile([P, S], dt.bfloat16, tag="ex")
                    nc.scalar.activation(ex[:], ps[:], AF.Exp,
                                         scale=scale / (FS * FS),
                                         accum_out=payload[:, t:t + 1])
                    if t == TB - 1:
                        # split the last accumulate so each half of the
                        # result DMA can start as soon as its half is done
                        nc.vector.tensor_add(acc[:, 0:H], acc[:, 0:H],
                                             ex[:, 0:H])
                        nc.vector.tensor_add(acc[:, H:], acc[:, H:],
                                             ex[:, H:])
                    else:
                        nc.vector.tensor_add(acc[:], acc[:], ex[:])

                    # diagonal partials on VectorE slack: one c-chunk of
                    # prod = a_sb * t_sb reduced over the free axis, every
                    # 8th block once t_sb has surely landed
                    if t >= 8 and t % 8 == 0 and (t // 8) <= NCH:
                        c = t // 8 - 1
                        prodc = gpool.tile([P, KT, CH], dt.bfloat16,
                                           tag="prod")
                        nc.vector.tensor_mul(prodc[:], a_sb[:, c], t_sb[:, c])
                        nc.vector.reduce_sum(dvec4[:, c:c + 1], prodc[:],
                                             axis=mybir.AxisListType.XY)
                        if c == NCH - 1:
                            # ready long before the loop ends; overlaps
                            nc.gpsimd.dma_start(out_dvec[:], dvec4[:])

            # ---- tail: just ship the partials ----
            nc.gpsimd.dma_start(out_col[:], payload[:])
            nc.sync.dma_start(out_acc[:, 0:H], acc[:, 0:H])
            nc.gpsimd.dma_start(out_acc[:, H:], acc[:, H:])

    nc.compile()
    return nc


_CACHE = {}


def _make_in_maps(img_f32, txt_f32):
    import concourse.mybir as mybir
    fp8 = mybir.dt.np(mybir.dt.float8e4)

    imgq = (img_f32 * FS).astype(fp8)
    txtq = (txt_f32 * FS).astype(fp8)

    # B[t, p, k, j] = txt[t*128+j, k*128+p]  (stationary operand tiles)
    Bm = np.ascontiguousarray(
        txtq.reshape(TB, P, KT, P).transpose(0, 3, 2, 1))

    def shard_T(x):  # [S, D] -> [p, c, k, ic] = x[c*CH+ic, k*128+p]
        return np.ascontiguousarray(
            x.reshape(NCH, CH, KT, P).transpose(3, 0, 2, 1))

    in_maps = []
    for c in range(NCORES):
        in_maps.append({
            "img_a": shard_T(imgq[c * S:(c + 1) * S]),
            "txt_t": shard_T(txtq[c * S:(c + 1) * S]),
            "txt_b": Bm,
        })
    return in_maps


def kernel(all_image_features, all_text_features, logit_scale, labels=None,
           **_unused):
    from concourse import bass_utils

    img = np.asarray(all_image_features, dtype=np.float32)
    txt = np.asarray(all_text_features, dtype=np.float32)
    scale = float(np.asarray(logit_scale))

    if scale not in _CACHE:
        _CACHE[scale] = _build(scale)
    nc = _CACHE[scale]

    in_maps = _make_in_maps(img, txt)
    res = bass_utils.run_bass_kernel_spmd(nc, in_maps,
                                          core_ids=list(range(NCORES)))

    # host-side unshard: O(N) combine of the per-core partials
    colsum = np.zeros((P, TB), dtype=np.float64)
    rowlse = 0.0
    diag = 0.0
    for c in range(NCORES):
        r = res.results[c]
        colsum += np.asarray(r["colp"], dtype=np.float64)
        rows = np.asarray(r["accout"]).astype(np.float64).sum(axis=0)
        rowlse += np.log(rows + EPS).sum()
        diag += float(np.asarray(r["dvec"], dtype=np.float64).sum())
    collse = np.log(colsum + EPS).sum()
    loss = (rowlse + collse) / (2.0 * N) - scale * diag / (N * FS * FS)
    return np.float32(loss)
